# revision 1
# baseline (speedup 1.0000x reference)
"""2-layer GAT on 8 Trainium2 NeuronCores.

Strategy
--------
Core c owns destination nodes [c*12500, (c+1)*12500); every edge lives on the
core that owns its destination, so the scatter-softmax segment reduction is
entirely core-local. Between layers, only a small AllGather of per-node
feature tables ([H | alpha_src | 1] rows) crosses cores.

Per core, destination nodes are bucketed by local in-degree padded to a
multiple of 8 (R). Each node owns exactly R contiguous "slot" columns in a
[128, T_U] slot grid; real edges fill the first deg slots, the rest point at
an all-zero dummy table row. This makes the segment softmax/reduction a set
of REGULAR strided DVE ops (no scatter at all). The only irreducible random
access is the per-slot gather of table rows Haug[src], done with one
indirect DMA per chunk (per-index descriptors).

Softmax is computed without the max-subtraction: attention logits here are
bounded (|e| < ~10) so exp() is safe in fp32, and the result is identical to
the reference's stabilized form up to rounding. Normalization happens once
per node after the weighted sum: out = U[:, :D] / (U[:, D+1] + 1e-16).

The Bass program is built AFTER seeing the inputs (shapes/layouts baked in),
compiled via neuronx-cc, and run SPMD on 8 cores through the PJRT path.
"""
import sys

sys.path.insert(0, "/opt/trn_rl_repo")

import numpy as np

P = 128
N_NODES = 100000
N_CORES = 8
IN_DIM = 256
HID = 8
OUT = 16
NEG = 0.2


class _Meta:
    pass


def _preprocess(E, X, RQ=4, target_chunk=512):
    N, C = N_NODES, N_CORES
    NLOC = N // C
    src = np.asarray(E[0], dtype=np.int64)
    dst = np.asarray(E[1], dtype=np.int64)

    deg = np.zeros((C, NLOC), dtype=np.int64)
    np.add.at(deg.reshape(-1), dst, 1)


    # Data-adaptive bucket boundaries (DP): minimize total slot columns
    # sum_b nrow_b * R_b over degree-range buckets, instead of fixed RQ.
    dmax = int(deg.max())
    cntd = np.zeros((C, dmax + 1), dtype=np.int64)
    for c in range(C):
        cntd[c] = np.bincount(deg[c][deg[c] > 0], minlength=dmax + 1)
    pred = cntd.cumsum(axis=1)
    INF = 1 << 60
    fdp = [0] + [INF] * dmax
    chx = [0] * (dmax + 1)
    for j in range(1, dmax + 1):
        for i in range(1, j + 1):
            n = pred[:, j] - pred[:, i - 1]
            v = fdp[i - 1] + int(np.ceil(n.max() / P)) * j
            if v < fdp[j]:
                fdp[j] = v
                chx[j] = i
    deg2R = np.zeros(dmax + 1, dtype=np.int64)
    j = dmax
    while j > 0:
        i = chx[j]
        deg2R[i:j + 1] = j
        j = i - 1
    Rv = deg2R[deg]

    Rs = sorted(set(int(r) for r in np.unique(Rv) if r > 0))
    has_zero = bool((Rv == 0).any())
    Rs_cells = Rs + ([0] if has_zero else [])

    nrow = {}
    for R in Rs_cells:
        cnt = (Rv == R).sum(axis=1)
        nrow[R] = int(np.ceil(cnt.max() / P))
    nrow_tot = sum(nrow.values()) + 1
    NR = P * nrow_tot
    DUMMY = C * NR - 1

    colbase = {}
    cb = 0
    for R in Rs_cells:
        colbase[R] = cb
        cb += nrow[R]
    slotbase = {}
    sb = 0
    for R in Rs:
        slotbase[R] = sb
        sb += nrow[R] * R
    T_U = sb

    meta = _Meta()
    meta.N, meta.C, meta.NLOC, meta.NR = N, C, NLOC, NR
    meta.nrow_tot, meta.T_U, meta.Rs = nrow_tot, T_U, Rs
    meta.nrow, meta.colbase, meta.slotbase = nrow, colbase, slotbase
    meta.DUMMY = DUMMY

    cell2node = np.full((C, P, nrow_tot), -1, dtype=np.int64)
    tabrow_of = np.full(N, DUMMY, dtype=np.int64)
    for c in range(C):
        for R in Rs_cells:
            ls = np.nonzero(Rv[c] == R)[0]
            k = np.arange(len(ls))
            p = k % P
            i = colbase[R] + k // P
            cell2node[c, p, i] = ls
            tabrow_of[c * NLOC + ls] = c * NR + p * nrow_tot + i
    meta.cell2node = cell2node
    meta.ones = (cell2node >= 0).astype(np.float32)

    idx = np.full((C, P, T_U), DUMMY, dtype=np.int64)
    order = np.argsort(dst, kind="stable")
    s_src = src[order]
    s_dst = dst[order]
    grp_start = np.searchsorted(s_dst, np.arange(N))
    pos_in_grp = np.arange(len(s_dst)) - grp_start[s_dst]
    e_c = s_dst // NLOC
    e_l = s_dst % NLOC
    e_R = Rv[e_c, e_l]
    e_k = np.zeros(len(s_dst), dtype=np.int64)
    for c in range(C):
        for R in Rs:
            ls = np.nonzero(Rv[c] == R)[0]
            rank = np.full(NLOC, -1, dtype=np.int64)
            rank[ls] = np.arange(len(ls))
            m = (e_c == c) & (e_R == R)
            e_k[m] = rank[e_l[m]]
    e_p = e_k % P
    e_i_off = e_k // P
    sb_arr = np.array([slotbase[int(r)] for r in e_R])
    e_t = sb_arr + e_i_off * e_R + pos_in_grp
    idx[e_c, e_p, e_t] = tabrow_of[s_src]
    meta.idx = idx.astype(np.int32)

    XcT = np.zeros((C, IN_DIM, NR), dtype=np.float32)
    Xf = np.asarray(X, np.float32)
    for c in range(C):
        pp, ii = np.nonzero(cell2node[c] >= 0)
        ls = cell2node[c, pp, ii]
        XcT[c][:, ii * P + pp] = Xf[c * NLOC + ls].T
    meta.XcT = XcT

    col_R = np.zeros(nrow_tot, dtype=np.int64)
    col_sb = np.zeros(nrow_tot + 1, dtype=np.int64)
    for R in Rs:
        col_R[colbase[R]:colbase[R] + nrow[R]] = R
    acc = 0
    for i in range(nrow_tot):
        col_sb[i] = acc
        acc += col_R[i]
    col_sb[nrow_tot] = acc
    chunks = []
    i0 = 0
    while i0 < nrow_tot and col_R[i0] > 0:
        i1 = i0
        while (i1 < nrow_tot and col_R[i1] > 0
               and col_sb[i1 + 1] - col_sb[i0] <= target_chunk):
            i1 += 1
        inters = []
        for R in Rs:
            ia = max(i0, colbase[R])
            ib = min(i1, colbase[R] + nrow[R])
            if ia < ib:
                inters.append((R, ia, ib, int(col_sb[ia])))
        chunks.append((i0, i1, int(col_sb[i0]), int(col_sb[i1]), inters))
        i0 = i1
    meta.chunks = chunks
    return meta


def _build(meta):
    import concourse.bass as bass
    import concourse.bacc as bacc
    import concourse.mybir as mybir
    import concourse.tile as tile

    F32 = mybir.dt.float32
    I32 = mybir.dt.int32
    AX = mybir.AxisListType
    OP = mybir.AluOpType
    AF = mybir.ActivationFunctionType

    C, NR, nt, T_U = meta.C, meta.NR, meta.nrow_tot, meta.T_U
    D1, D2 = HID + 2, OUT + 2
    KCH = IN_DIM // P

    nc = bacc.Bacc()
    XcT_d = nc.declare_dram_parameter("XcT", [IN_DIM, NR], F32, isOutput=False)
    ones_d = nc.declare_dram_parameter("ones", [P, nt], F32, isOutput=False)
    idx_d = nc.declare_dram_parameter("idx", [P, T_U], I32, isOutput=False)
    W1_d = nc.declare_dram_parameter("W1", [IN_DIM, HID], F32, isOutput=False)
    a1s_d = nc.declare_dram_parameter("a1s", [1, HID], F32, isOutput=False)
    a1d_d = nc.declare_dram_parameter("a1d", [1, HID], F32, isOutput=False)
    W2T_d = nc.declare_dram_parameter("W2T", [1, OUT * HID], F32, isOutput=False)
    a2s_d = nc.declare_dram_parameter("a2s", [1, OUT], F32, isOutput=False)
    a2d_d = nc.declare_dram_parameter("a2d", [1, OUT], F32, isOutput=False)
    out_d = nc.declare_dram_parameter("out", [P, nt * OUT], F32, isOutput=True)

    cc1_d = nc.dram_tensor("cc1", [P, nt * D1], F32)
    tab1_d = nc.dram_tensor("tab1", [C * NR, D1], F32, addr_space="Shared")
    cc2_d = nc.dram_tensor("cc2", [P, nt * D2], F32)
    tab2_d = nc.dram_tensor("tab2", [C * NR, D2], F32, addr_space="Shared")
    groups = [list(range(C))]

    with tile.TileContext(nc) as tc:
        with (
            tc.tile_pool(name="persist", bufs=1) as pp,
            tc.tile_pool(name="xs", bufs=2) as xp,
            tc.tile_pool(name="gp", bufs=2) as gp,
            tc.tile_pool(name="ew", bufs=2) as ewp,
            tc.tile_pool(name="tmp", bufs=1) as tp,
            tc.tile_pool(name="ps", bufs=8, space="PSUM") as psp,
        ):
            idx_t = pp.tile([P, T_U], I32, tag="idx")
            nc.sync.dma_start(out=idx_t[:], in_=idx_d[:])
            ones_t = pp.tile([P, nt], F32, tag="ones")
            nc.sync.dma_start(out=ones_t[:], in_=ones_d[:])
            w1_t = pp.tile([P, KCH * HID], F32, tag="w1")
            for k in range(KCH):
                nc.sync.dma_start(out=w1_t[:, k * HID:(k + 1) * HID],
                                  in_=W1_d[k * P:(k + 1) * P, :])
            a1s_t = pp.tile([P, HID], F32, tag="a1s")
            nc.sync.dma_start(out=a1s_t[:], in_=a1s_d[0:1, :].to_broadcast([P, HID]))
            a1d_t = pp.tile([P, HID], F32, tag="a1d")
            nc.sync.dma_start(out=a1d_t[:], in_=a1d_d[0:1, :].to_broadcast([P, HID]))
            w2t_t = pp.tile([P, OUT * HID], F32, tag="w2t")
            nc.sync.dma_start(out=w2t_t[:],
                              in_=W2T_d[0:1, :].to_broadcast([P, OUT * HID]))
            a2s_t = pp.tile([P, OUT], F32, tag="a2s")
            nc.sync.dma_start(out=a2s_t[:], in_=a2s_d[0:1, :].to_broadcast([P, OUT]))
            a2d_t = pp.tile([P, OUT], F32, tag="a2d")
            nc.sync.dma_start(out=a2d_t[:], in_=a2d_d[0:1, :].to_broadcast([P, OUT]))

            # layer-1 node table: H = X @ W1 per 128-node block
            hg = pp.tile([P, nt * D1], F32, tag="hg")
            XB = 8
            for b0 in range(0, nt, XB):
                b1 = min(b0 + XB, nt)
                nb = b1 - b0
                xt = xp.tile([P, KCH * XB * P], F32, tag="xt")
                for k in range(KCH):
                    nc.sync.dma_start(out=xt[:, k * XB * P:k * XB * P + nb * P],
                                      in_=XcT_d[k * P:(k + 1) * P, b0 * P:b1 * P])
                for b in range(b0, b1):
                    ps = psp.tile([P, HID], F32, tag="hps")
                    for k in range(KCH):
                        nc.tensor.matmul(
                            out=ps[:],
                            lhsT=xt[:, k * XB * P + (b - b0) * P:
                                    k * XB * P + (b - b0 + 1) * P],
                            rhs=w1_t[:, k * HID:(k + 1) * HID],
                            start=(k == 0), stop=(k == KCH - 1))
                    nc.scalar.copy(out=hg[:, b * D1:b * D1 + HID], in_=ps[:])
            hv = hg[:].rearrange("p (n j) -> p n j", j=D1)[:, :, 0:HID]
            t_a = tp.tile([P, nt * HID], F32, tag="amul")
            tv = t_a[:].rearrange("p (n j) -> p n j", j=HID)
            nc.vector.tensor_tensor(out=tv, in0=hv,
                                    in1=a1s_t[:, None, :].to_broadcast([P, nt, HID]),
                                    op=OP.mult)
            nc.vector.tensor_reduce(
                out=hg[:].rearrange("p (n j) -> p n j", j=D1)[:, :, HID:HID + 1],
                in_=tv[:, :, None, :], axis=AX.X, op=OP.add)
            ad1_t = pp.tile([P, nt], F32, tag="ad1")
            t_b = tp.tile([P, nt * HID], F32, tag="amul")
            tv2 = t_b[:].rearrange("p (n j) -> p n j", j=HID)
            nc.vector.tensor_tensor(out=tv2, in0=hv,
                                    in1=a1d_t[:, None, :].to_broadcast([P, nt, HID]),
                                    op=OP.mult)
            nc.vector.tensor_reduce(out=ad1_t[:, :, None], in_=tv2[:, :, None, :],
                                    axis=AX.X, op=OP.add)
            nc.vector.tensor_copy(
                out=hg[:].rearrange("p (n j) -> p n j", j=D1)[:, :, HID + 1:HID + 2],
                in_=ones_t[:, :, None])
            nc.sync.dma_start(out=cc1_d[:], in_=hg[:])
            nc.gpsimd.collective_compute(
                "AllGather", OP.bypass, replica_groups=groups,
                ins=[cc1_d[:]], outs=[tab1_d[:]])

            def edge_layer(tab_d, D, ad_t, U):
                for (i0, i1, s0, s1, inters) in meta.chunks:
                    SC = s1 - s0
                    g_t = gp.tile([P, SC * D2], F32, tag="g")
                    gD = g_t[:, :SC * D]
                    # HW indirect DMA supports exactly one offset per
                    # partition with a 2D dest: one instruction per slot col.
                    for t in range(s0, s1):
                        nc.gpsimd.indirect_dma_start(
                            out=g_t[:, (t - s0) * D:(t - s0 + 1) * D],
                            out_offset=None, in_=tab_d[:],
                            in_offset=bass.IndirectOffsetOnAxis(
                                ap=idx_t[:, t:t + 1], axis=0))
                    e_t = ewp.tile([P, SC], F32, tag="e")
                    w_t = ewp.tile([P, SC], F32, tag="w")
                    for (R, ia, ib, sa) in inters:
                        nn = ib - ia
                        o = sa - s0
                        ev = e_t[:, o:o + nn * R].rearrange("p (n r) -> p n r", r=R)
                        gv = gD[:, o * D:(o + nn * R) * D].rearrange(
                            "p (n r j) -> p n r j", r=R, j=D)[:, :, :, D - 2]
                        adv = ad_t[:, ia:ib, None].to_broadcast([P, nn, R])
                        nc.vector.tensor_tensor(out=ev, in0=gv, in1=adv, op=OP.add)
                    nc.vector.tensor_scalar_mul(w_t[:], e_t[:], NEG)
                    nc.vector.tensor_tensor(out=w_t[:], in0=w_t[:], in1=e_t[:],
                                            op=OP.max)
                    nc.scalar.activation(w_t[:], w_t[:], AF.Exp)
                    nc.vector.tensor_tensor(
                        out=gD.rearrange("p (s j) -> p s j", j=D),
                        in0=gD.rearrange("p (s j) -> p s j", j=D),
                        in1=w_t[:, :, None].to_broadcast([P, SC, D]), op=OP.mult)
                    for (R, ia, ib, sa) in inters:
                        nn = ib - ia
                        o = sa - s0
                        uv = U[:, ia * D:ib * D].rearrange("p (n j) -> p n j", j=D)
                        gv = gD[:, o * D:(o + nn * R) * D].rearrange(
                            "p (n r j) -> p n j r", r=R, j=D)
                        nc.vector.tensor_reduce(out=uv, in_=gv, axis=AX.X, op=OP.add)

            U1 = pp.tile([P, nt * D1], F32, tag="U1")
            nc.vector.memset(U1[:], 0.0)
            edge_layer(tab1_d, D1, ad1_t, U1)

            z_t = tp.tile([P, nt], F32, tag="z")
            nc.vector.tensor_scalar_add(
                z_t[:, :, None],
                U1[:].rearrange("p (n j) -> p n j", j=D1)[:, :, D1 - 1:D1], 1e-16)
            rec_t = tp.tile([P, nt], F32, tag="rec")
            nc.vector.reciprocal(rec_t[:], z_t[:])
            h2 = pp.tile([P, nt * HID], F32, tag="h2")
            h2v = h2[:].rearrange("p (n j) -> p n j", j=HID)
            nc.vector.tensor_tensor(
                out=h2v, in0=U1[:].rearrange("p (n j) -> p n j", j=D1)[:, :, 0:HID],
                in1=rec_t[:, :, None].to_broadcast([P, nt, HID]), op=OP.mult)
            tneg = tp.tile([P, nt * HID], F32, tag="telu")
            nc.vector.tensor_scalar_min(tneg[:], h2[:], 0.0)
            nc.scalar.activation(tneg[:], tneg[:], AF.Exp)
            nc.vector.tensor_scalar_max(h2[:], h2[:], 0.0)
            nc.vector.tensor_tensor(out=h2[:], in0=h2[:], in1=tneg[:], op=OP.add)
            nc.vector.tensor_scalar_add(h2[:], h2[:], -1.0)
            hg2 = pp.tile([P, nt * D2], F32, tag="hg2")
            CB = 16
            for c0 in range(0, nt, CB):
                c1 = min(c0 + CB, nt)
                nn = c1 - c0
                tmw = tp.tile([P, CB * OUT * HID], F32, tag="tmw")
                tmv = tmw[:, :nn * OUT * HID].rearrange(
                    "p (n o j) -> p n o j", o=OUT, j=HID)
                nc.vector.tensor_tensor(
                    out=tmv,
                    in0=h2[:, c0 * HID:c1 * HID].rearrange(
                        "p (n j) -> p n j", j=HID)[:, :, None, :].to_broadcast(
                        [P, nn, OUT, HID]),
                    in1=w2t_t[:, None, :].to_broadcast(
                        [P, nn, OUT * HID]).rearrange("p n (o j) -> p n o j", o=OUT),
                    op=OP.mult)
                nc.vector.tensor_reduce(
                    out=hg2[:, c0 * D2:c1 * D2].rearrange(
                        "p (n j) -> p n j", j=D2)[:, :, 0:OUT],
                    in_=tmv, axis=AX.X, op=OP.add)
            hg2v = hg2[:].rearrange("p (n j) -> p n j", j=D2)
            ad2_t = pp.tile([P, nt], F32, tag="ad2")
            for (vec_t, dest) in ((a2s_t, hg2v[:, :, OUT:OUT + 1]),
                                  (a2d_t, ad2_t[:, :, None])):
                t_c = tp.tile([P, nt * OUT], F32, tag="amul2")
                tv3 = t_c[:].rearrange("p (n j) -> p n j", j=OUT)
                nc.vector.tensor_tensor(
                    out=tv3, in0=hg2v[:, :, 0:OUT],
                    in1=vec_t[:, None, :].to_broadcast([P, nt, OUT]), op=OP.mult)
                nc.vector.tensor_reduce(out=dest, in_=tv3[:, :, None, :],
                                        axis=AX.X, op=OP.add)
            nc.vector.tensor_copy(out=hg2v[:, :, OUT + 1:OUT + 2],
                                  in_=ones_t[:, :, None])
            nc.sync.dma_start(out=cc2_d[:], in_=hg2[:])
            nc.gpsimd.collective_compute(
                "AllGather", OP.bypass, replica_groups=groups,
                ins=[cc2_d[:]], outs=[tab2_d[:]])

            U2 = pp.tile([P, nt * D2], F32, tag="U2")
            nc.vector.memset(U2[:], 0.0)
            edge_layer(tab2_d, D2, ad2_t, U2)

            z2_t = tp.tile([P, nt], F32, tag="z")
            nc.vector.tensor_scalar_add(
                z2_t[:, :, None],
                U2[:].rearrange("p (n j) -> p n j", j=D2)[:, :, D2 - 1:D2], 1e-16)
            rec2_t = tp.tile([P, nt], F32, tag="rec")
            nc.vector.reciprocal(rec2_t[:], z2_t[:])
            o_t = pp.tile([P, nt * OUT], F32, tag="out")
            nc.vector.tensor_tensor(
                out=o_t[:].rearrange("p (n j) -> p n j", j=OUT),
                in0=U2[:].rearrange("p (n j) -> p n j", j=D2)[:, :, 0:OUT],
                in1=rec2_t[:, :, None].to_broadcast([P, nt, OUT]), op=OP.mult)
            nc.sync.dma_start(out=out_d[:], in_=o_t[:])
    nc.finalize()
    return nc


def kernel(V, E, X, W1, a1_src, a1_dst, W2, a2_src, a2_dst):
    meta = _preprocess(E, X)
    nc = _build(meta)

    from concourse.bass_utils import run_bass_kernel_spmd

    in_maps = []
    for c in range(N_CORES):
        in_maps.append({
            "XcT": np.ascontiguousarray(meta.XcT[c]),
            "ones": np.ascontiguousarray(meta.ones[c]),
            "idx": np.ascontiguousarray(meta.idx[c]),
            "W1": np.asarray(W1, np.float32),
            "a1s": np.asarray(a1_src, np.float32).reshape(1, -1),
            "a1d": np.asarray(a1_dst, np.float32).reshape(1, -1),
            "W2T": np.ascontiguousarray(np.asarray(W2, np.float32).T).reshape(1, -1),
            "a2s": np.asarray(a2_src, np.float32).reshape(1, -1),
            "a2d": np.asarray(a2_dst, np.float32).reshape(1, -1),
        })
    res = run_bass_kernel_spmd(nc, in_maps, list(range(N_CORES)))

    out = np.zeros((N_NODES, OUT), dtype=np.float32)
    for c in range(N_CORES):
        g = res.results[c]["out"].reshape(P, meta.nrow_tot, OUT)
        pp, ii = np.nonzero(meta.cell2node[c] >= 0)
        ls = meta.cell2node[c, pp, ii]
        out[c * meta.NLOC + ls] = g[pp, ii]
    return out



# revision 18
# speedup vs baseline: 1.0970x; 1.0970x over previous
"""2-layer GAT on 8 Trainium2 NeuronCores.

Strategy
--------
Core c owns destination nodes [c*12500, (c+1)*12500); every edge lives on the
core that owns its destination, so the scatter-softmax segment reduction is
entirely core-local. Between layers, only a small AllGather of per-node
feature tables ([H | alpha_src | 1] rows) crosses cores.

Per core, destination nodes are bucketed by local in-degree padded to a
multiple of 8 (R). Each node owns exactly R contiguous "slot" columns in a
[128, T_U] slot grid; real edges fill the first deg slots, the rest point at
an all-zero dummy table row. This makes the segment softmax/reduction a set
of REGULAR strided DVE ops (no scatter at all). The only irreducible random
access is the per-slot gather of table rows Haug[src], done with one
indirect DMA per chunk (per-index descriptors).

Softmax is computed without the max-subtraction: attention logits here are
bounded (|e| < ~10) so exp() is safe in fp32, and the result is identical to
the reference's stabilized form up to rounding. Normalization happens once
per node after the weighted sum: out = U[:, :D] / (U[:, D+1] + 1e-16).

The Bass program is built AFTER seeing the inputs (shapes/layouts baked in),
compiled via neuronx-cc, and run SPMD on 8 cores through the PJRT path.
"""
import sys

sys.path.insert(0, "/opt/trn_rl_repo")

import ml_dtypes
import numpy as np

P = 128
N_NODES = 100000
N_CORES = 8
IN_DIM = 256
HID = 8
OUT = 16
NEG = 0.2


class _Meta:
    pass


def _preprocess(E, X, RQ=4, target_chunk=512):
    N, C = N_NODES, N_CORES
    NLOC = N // C
    src = np.asarray(E[0], dtype=np.int64)
    dst_orig = np.asarray(E[1], dtype=np.int64)

    # Degree-balanced node->core assignment (snake-deal by in-degree) so the
    # per-bucket node counts match across cores and the shared slot grid has
    # minimal cross-core padding. p2n[new_id] = original node id.
    deg_glob = np.bincount(dst_orig, minlength=N)
    order = np.argsort(deg_glob, kind="stable")
    g = np.arange(N) // C
    r = np.arange(N) % C
    core_of_sorted = np.where(g % 2 == 0, r, C - 1 - r)
    p2n = np.zeros(N, dtype=np.int64)
    fill = np.zeros(C, dtype=np.int64)
    for i in range(N):
        c = core_of_sorted[i]
        p2n[c * NLOC + fill[c]] = order[i]
        fill[c] += 1
    n2p = np.zeros(N, dtype=np.int64)
    n2p[p2n] = np.arange(N)
    dst = n2p[dst_orig]

    deg = np.zeros((C, NLOC), dtype=np.int64)
    np.add.at(deg.reshape(-1), dst, 1)


    # Data-adaptive bucket boundaries (DP): minimize total slot columns
    # sum_b nrow_b * R_b over degree-range buckets, instead of fixed RQ.
    dmax = int(deg.max())
    cntd = np.zeros((C, dmax + 1), dtype=np.int64)
    for c in range(C):
        cntd[c] = np.bincount(deg[c][deg[c] > 0], minlength=dmax + 1)
    pred = cntd.cumsum(axis=1)
    INF = 1 << 60
    fdp = [0] + [INF] * dmax
    chx = [0] * (dmax + 1)
    for j in range(1, dmax + 1):
        for i in range(1, j + 1):
            n = pred[:, j] - pred[:, i - 1]
            v = fdp[i - 1] + int(np.ceil(n.max() / P)) * j
            if v < fdp[j]:
                fdp[j] = v
                chx[j] = i
    deg2R = np.zeros(dmax + 1, dtype=np.int64)
    j = dmax
    while j > 0:
        i = chx[j]
        deg2R[i:j + 1] = j
        j = i - 1
    Rv = deg2R[deg]

    Rs = sorted(set(int(r) for r in np.unique(Rv) if r > 0))
    has_zero = bool((Rv == 0).any())
    Rs_cells = Rs + ([0] if has_zero else [])

    nrow = {}
    for R in Rs_cells:
        cnt = (Rv == R).sum(axis=1)
        nrow[R] = int(np.ceil(cnt.max() / P))
    nrow_tot = sum(nrow.values()) + 1
    NR = P * nrow_tot
    DUMMY = C * NR - 1

    colbase = {}
    cb = 0
    for R in Rs_cells:
        colbase[R] = cb
        cb += nrow[R]
    slotbase = {}
    sb = 0
    for R in Rs:
        slotbase[R] = sb
        sb += nrow[R] * R
    T_U = sb

    meta = _Meta()
    meta.N, meta.C, meta.NLOC, meta.NR = N, C, NLOC, NR
    meta.nrow_tot, meta.T_U, meta.Rs = nrow_tot, T_U, Rs
    meta.nrow, meta.colbase, meta.slotbase = nrow, colbase, slotbase
    meta.DUMMY = DUMMY

    cell2node = np.full((C, P, nrow_tot), -1, dtype=np.int64)
    tabrow_of = np.full(N, DUMMY, dtype=np.int64)
    for c in range(C):
        for R in Rs_cells:
            ls = np.nonzero(Rv[c] == R)[0]
            k = np.arange(len(ls))
            p = k % P
            i = colbase[R] + k // P
            cell2node[c, p, i] = ls
            tabrow_of[p2n[c * NLOC + ls]] = c * NR + p * nrow_tot + i
    meta.cell2node = cell2node
    meta.ones = (cell2node >= 0).astype(np.float32)

    idx = np.full((C, P, T_U), DUMMY, dtype=np.int64)
    order = np.argsort(dst, kind="stable")
    s_src = src[order]
    s_dst = dst[order]
    grp_start = np.searchsorted(s_dst, np.arange(N))
    pos_in_grp = np.arange(len(s_dst)) - grp_start[s_dst]
    e_c = s_dst // NLOC
    e_l = s_dst % NLOC
    e_R = Rv[e_c, e_l]
    e_k = np.zeros(len(s_dst), dtype=np.int64)
    for c in range(C):
        for R in Rs:
            ls = np.nonzero(Rv[c] == R)[0]
            rank = np.full(NLOC, -1, dtype=np.int64)
            rank[ls] = np.arange(len(ls))
            m = (e_c == c) & (e_R == R)
            e_k[m] = rank[e_l[m]]
    e_p = e_k % P
    e_i_off = e_k // P
    sb_arr = np.array([slotbase[int(r)] for r in e_R])
    e_t = sb_arr + e_i_off * e_R + pos_in_grp
    idx[e_c, e_p, e_t] = tabrow_of[s_src]
    meta.idx = idx.astype(np.int32)

    XcT = np.zeros((C, IN_DIM, NR), dtype=np.float32)
    Xf = np.asarray(X, np.float32)
    for c in range(C):
        pp, ii = np.nonzero(cell2node[c] >= 0)
        ls = cell2node[c, pp, ii]
        XcT[c][:, ii * P + pp] = Xf[p2n[c * NLOC + ls]].T
    meta.XcT = XcT.astype(ml_dtypes.bfloat16)
    meta.p2n = p2n

    col_R = np.zeros(nrow_tot, dtype=np.int64)
    col_sb = np.zeros(nrow_tot + 1, dtype=np.int64)
    for R in Rs:
        col_R[colbase[R]:colbase[R] + nrow[R]] = R
    acc = 0
    for i in range(nrow_tot):
        col_sb[i] = acc
        acc += col_R[i]
    col_sb[nrow_tot] = acc
    chunks = []
    i0 = 0
    while i0 < nrow_tot and col_R[i0] > 0:
        i1 = i0
        while (i1 < nrow_tot and col_R[i1] > 0
               and col_sb[i1 + 1] - col_sb[i0] <= target_chunk):
            i1 += 1
        inters = []
        for R in Rs:
            ia = max(i0, colbase[R])
            ib = min(i1, colbase[R] + nrow[R])
            if ia < ib:
                inters.append((R, ia, ib, int(col_sb[ia])))
        chunks.append((i0, i1, int(col_sb[i0]), int(col_sb[i1]), inters))
        i0 = i1
    meta.chunks = chunks
    return meta


def _build(meta):
    import concourse.bass as bass
    import concourse.bacc as bacc
    import concourse.mybir as mybir
    import concourse.tile as tile

    F32 = mybir.dt.float32
    I32 = mybir.dt.int32
    AX = mybir.AxisListType
    OP = mybir.AluOpType
    AF = mybir.ActivationFunctionType

    C, NR, nt, T_U = meta.C, meta.NR, meta.nrow_tot, meta.T_U
    D1, D2 = HID + 2, OUT + 2
    KCH = IN_DIM // P

    nc = bacc.Bacc()
    BF16 = mybir.dt.bfloat16
    XcT_d = nc.declare_dram_parameter("XcT", [IN_DIM, NR], BF16, isOutput=False)
    ones_d = nc.declare_dram_parameter("ones", [P, nt], F32, isOutput=False)
    idx_d = nc.declare_dram_parameter("idx", [P, T_U], I32, isOutput=False)
    W1_d = nc.declare_dram_parameter("W1", [IN_DIM, HID], BF16, isOutput=False)
    a1s_d = nc.declare_dram_parameter("a1s", [1, HID], F32, isOutput=False)
    a1d_d = nc.declare_dram_parameter("a1d", [1, HID], F32, isOutput=False)
    W2T_d = nc.declare_dram_parameter("W2T", [1, OUT * HID], F32, isOutput=False)
    a2s_d = nc.declare_dram_parameter("a2s", [1, OUT], F32, isOutput=False)
    a2d_d = nc.declare_dram_parameter("a2d", [1, OUT], F32, isOutput=False)
    out_d = nc.declare_dram_parameter("out", [P, nt * OUT], F32, isOutput=True)

    cc1_d = nc.dram_tensor("cc1", [P, nt * D1], F32)
    tab1_d = nc.dram_tensor("tab1", [C * NR, D1], F32, addr_space="Shared")
    cc2_d = nc.dram_tensor("cc2", [P, nt * D2], F32)
    tab2_d = nc.dram_tensor("tab2", [C * NR, D2], F32, addr_space="Shared")
    groups = [list(range(C))]

    with tile.TileContext(nc) as tc:
        with (
            tc.tile_pool(name="persist", bufs=1) as pp,
            tc.tile_pool(name="xs", bufs=2) as xp,
            tc.tile_pool(name="gp", bufs=2) as gp,
            tc.tile_pool(name="ew", bufs=2) as ewp,
            tc.tile_pool(name="tmp", bufs=1) as tp,
            tc.tile_pool(name="ps", bufs=8, space="PSUM") as psp,
        ):
            idx_t = pp.tile([P, T_U], I32, tag="idx")
            nc.sync.dma_start(out=idx_t[:], in_=idx_d[:])
            ones_t = pp.tile([P, nt], F32, tag="ones")
            nc.sync.dma_start(out=ones_t[:], in_=ones_d[:])
            w1_t = pp.tile([P, KCH * HID], BF16, tag="w1")
            for k in range(KCH):
                nc.sync.dma_start(out=w1_t[:, k * HID:(k + 1) * HID],
                                  in_=W1_d[k * P:(k + 1) * P, :])
            a1s_t = pp.tile([P, HID], F32, tag="a1s")
            nc.sync.dma_start(out=a1s_t[:], in_=a1s_d[0:1, :].to_broadcast([P, HID]))
            a1d_t = pp.tile([P, HID], F32, tag="a1d")
            nc.sync.dma_start(out=a1d_t[:], in_=a1d_d[0:1, :].to_broadcast([P, HID]))
            w2t_t = pp.tile([P, OUT * HID], F32, tag="w2t")
            nc.sync.dma_start(out=w2t_t[:],
                              in_=W2T_d[0:1, :].to_broadcast([P, OUT * HID]))
            a2s_t = pp.tile([P, OUT], F32, tag="a2s")
            nc.sync.dma_start(out=a2s_t[:], in_=a2s_d[0:1, :].to_broadcast([P, OUT]))
            a2d_t = pp.tile([P, OUT], F32, tag="a2d")
            nc.sync.dma_start(out=a2d_t[:], in_=a2d_d[0:1, :].to_broadcast([P, OUT]))

            # layer-1 node table: H = X @ W1 per 128-node block
            hg = pp.tile([P, nt * D1], F32, tag="hg")
            XB = 8
            for b0 in range(0, nt, XB):
                b1 = min(b0 + XB, nt)
                nb = b1 - b0
                xt = xp.tile([P, KCH * XB * P], BF16, tag="xt")
                for k in range(KCH):
                    nc.sync.dma_start(out=xt[:, k * XB * P:k * XB * P + nb * P],
                                      in_=XcT_d[k * P:(k + 1) * P, b0 * P:b1 * P])
                for b in range(b0, b1):
                    ps = psp.tile([P, HID], F32, tag="hps")
                    for k in range(KCH):
                        nc.tensor.matmul(
                            out=ps[:],
                            lhsT=xt[:, k * XB * P + (b - b0) * P:
                                    k * XB * P + (b - b0 + 1) * P],
                            rhs=w1_t[:, k * HID:(k + 1) * HID],
                            start=(k == 0), stop=(k == KCH - 1))
                    nc.scalar.copy(out=hg[:, b * D1:b * D1 + HID], in_=ps[:])
            hv = hg[:].rearrange("p (n j) -> p n j", j=D1)[:, :, 0:HID]
            t_a = tp.tile([P, nt * HID], F32, tag="amul")
            tv = t_a[:].rearrange("p (n j) -> p n j", j=HID)
            nc.vector.tensor_tensor(out=tv, in0=hv,
                                    in1=a1s_t[:, None, :].to_broadcast([P, nt, HID]),
                                    op=OP.mult)
            nc.vector.tensor_reduce(
                out=hg[:].rearrange("p (n j) -> p n j", j=D1)[:, :, HID:HID + 1],
                in_=tv[:, :, None, :], axis=AX.X, op=OP.add)
            ad1_t = pp.tile([P, nt], F32, tag="ad1")
            t_b = tp.tile([P, nt * HID], F32, tag="amul")
            tv2 = t_b[:].rearrange("p (n j) -> p n j", j=HID)
            nc.vector.tensor_tensor(out=tv2, in0=hv,
                                    in1=a1d_t[:, None, :].to_broadcast([P, nt, HID]),
                                    op=OP.mult)
            nc.vector.tensor_reduce(out=ad1_t[:, :, None], in_=tv2[:, :, None, :],
                                    axis=AX.X, op=OP.add)
            nc.vector.tensor_copy(
                out=hg[:].rearrange("p (n j) -> p n j", j=D1)[:, :, HID + 1:HID + 2],
                in_=ones_t[:, :, None])
            nc.sync.dma_start(out=cc1_d[:], in_=hg[:])
            nc.gpsimd.collective_compute(
                "AllGather", OP.bypass, replica_groups=groups,
                ins=[cc1_d[:]], outs=[tab1_d[:]])

            def edge_layer(tab_d, D, ad_t, U, post=None):
                for (i0, i1, s0, s1, inters) in meta.chunks:
                    SC = s1 - s0
                    g_t = gp.tile([P, SC * D2], F32, tag="g")
                    gD = g_t[:, :SC * D]
                    # HW indirect DMA supports exactly one offset per
                    # partition with a 2D dest: one instruction per slot col.
                    for t in range(s0, s1):
                        nc.gpsimd.indirect_dma_start(
                            out=g_t[:, (t - s0) * D:(t - s0 + 1) * D],
                            out_offset=None, in_=tab_d[:],
                            in_offset=bass.IndirectOffsetOnAxis(
                                ap=idx_t[:, t:t + 1], axis=0))
                    e_t = ewp.tile([P, SC], F32, tag="e")
                    w_t = ewp.tile([P, SC], F32, tag="w")
                    for (R, ia, ib, sa) in inters:
                        nn = ib - ia
                        o = sa - s0
                        ev = e_t[:, o:o + nn * R].rearrange("p (n r) -> p n r", r=R)
                        gv = gD[:, o * D:(o + nn * R) * D].rearrange(
                            "p (n r j) -> p n r j", r=R, j=D)[:, :, :, D - 2]
                        adv = ad_t[:, ia:ib, None].to_broadcast([P, nn, R])
                        nc.vector.tensor_tensor(out=ev, in0=gv, in1=adv, op=OP.add)
                    nc.vector.tensor_scalar_mul(w_t[:], e_t[:], NEG)
                    nc.vector.tensor_tensor(out=w_t[:], in0=w_t[:], in1=e_t[:],
                                            op=OP.max)
                    nc.scalar.activation(w_t[:], w_t[:], AF.Exp)
                    nc.vector.tensor_tensor(
                        out=gD.rearrange("p (s j) -> p s j", j=D),
                        in0=gD.rearrange("p (s j) -> p s j", j=D),
                        in1=w_t[:, :, None].to_broadcast([P, SC, D]), op=OP.mult)
                    for (R, ia, ib, sa) in inters:
                        nn = ib - ia
                        o = sa - s0
                        uv = U[:, ia * D:ib * D].rearrange("p (n j) -> p n j", j=D)
                        gv = gD[:, o * D:(o + nn * R) * D].rearrange(
                            "p (n r j) -> p n j r", r=R, j=D)
                        nc.vector.tensor_reduce(out=uv, in_=gv, axis=AX.X, op=OP.add)
                    if post is not None:
                        post(i0, i1)
                # columns never covered by chunks (zero-degree bucket tail)
                if post is not None and meta.chunks[-1][1] < nt:
                    post(meta.chunks[-1][1], nt)

            U1 = pp.tile([P, nt * D1], F32, tag="U1")
            nc.vector.memset(U1[:], 0.0)

            # mid-stage tiles, written in [a, b) column slices per chunk so
            # the work overlaps the remaining layer-1 gather stream
            z_t = tp.tile([P, nt], F32, tag="z")
            rec_t = tp.tile([P, nt], F32, tag="rec")
            h2 = pp.tile([P, nt * HID], F32, tag="h2")
            tneg = tp.tile([P, nt * HID], F32, tag="telu")
            hg2 = pp.tile([P, nt * D2], F32, tag="hg2")
            ad2_t = pp.tile([P, nt], F32, tag="ad2")
            t_c = tp.tile([P, nt * OUT], F32, tag="amul2")
            CB = 16

            def post_mid(a, b):
                nn_r = b - a
                u1v = U1[:, a * D1:b * D1].rearrange("p (n j) -> p n j", j=D1)
                nc.vector.tensor_scalar_add(
                    z_t[:, a:b, None], u1v[:, :, D1 - 1:D1], 1e-16)
                nc.vector.reciprocal(rec_t[:, a:b], z_t[:, a:b])
                h2s = h2[:, a * HID:b * HID]
                nc.vector.tensor_tensor(
                    out=h2s.rearrange("p (n j) -> p n j", j=HID),
                    in0=u1v[:, :, 0:HID],
                    in1=rec_t[:, a:b, None].to_broadcast([P, nn_r, HID]),
                    op=OP.mult)
                tns = tneg[:, a * HID:b * HID]
                nc.vector.tensor_scalar_min(tns, h2s, 0.0)
                nc.scalar.activation(tns, tns, AF.Exp)
                nc.vector.tensor_scalar_max(h2s, h2s, 0.0)
                nc.vector.tensor_tensor(out=h2s, in0=h2s, in1=tns, op=OP.add)
                nc.vector.tensor_scalar_add(h2s, h2s, -1.0)
                for c0 in range(a, b, CB):
                    c1 = min(c0 + CB, b)
                    nn = c1 - c0
                    tmw = tp.tile([P, CB * OUT * HID], F32, tag="tmw")
                    tmv = tmw[:, :nn * OUT * HID].rearrange(
                        "p (n o j) -> p n o j", o=OUT, j=HID)
                    nc.vector.tensor_tensor(
                        out=tmv,
                        in0=h2[:, c0 * HID:c1 * HID].rearrange(
                            "p (n j) -> p n j", j=HID)[:, :, None, :].to_broadcast(
                            [P, nn, OUT, HID]),
                        in1=w2t_t[:, None, :].to_broadcast(
                            [P, nn, OUT * HID]).rearrange(
                            "p n (o j) -> p n o j", o=OUT),
                        op=OP.mult)
                    nc.vector.tensor_reduce(
                        out=hg2[:, c0 * D2:c1 * D2].rearrange(
                            "p (n j) -> p n j", j=D2)[:, :, 0:OUT],
                        in_=tmv, axis=AX.X, op=OP.add)
                hg2v = hg2[:, a * D2:b * D2].rearrange("p (n j) -> p n j", j=D2)
                for (vec_t, dest) in ((a2s_t, hg2v[:, :, OUT:OUT + 1]),
                                      (a2d_t, ad2_t[:, a:b, None])):
                    tv3 = t_c[:, a * OUT:b * OUT].rearrange(
                        "p (n j) -> p n j", j=OUT)
                    nc.vector.tensor_tensor(
                        out=tv3, in0=hg2v[:, :, 0:OUT],
                        in1=vec_t[:, None, :].to_broadcast([P, nn_r, OUT]),
                        op=OP.mult)
                    nc.vector.tensor_reduce(out=dest, in_=tv3[:, :, None, :],
                                            axis=AX.X, op=OP.add)
                nc.vector.tensor_copy(out=hg2v[:, :, OUT + 1:OUT + 2],
                                      in_=ones_t[:, a:b, None])
                nc.sync.dma_start(out=cc2_d[:, a * D2:b * D2],
                                  in_=hg2[:, a * D2:b * D2])

            edge_layer(tab1_d, D1, ad1_t, U1, post=post_mid)
            nc.gpsimd.collective_compute(
                "AllGather", OP.bypass, replica_groups=groups,
                ins=[cc2_d[:]], outs=[tab2_d[:]])

            U2 = pp.tile([P, nt * D2], F32, tag="U2")
            nc.vector.memset(U2[:], 0.0)
            o_t = pp.tile([P, nt * OUT], F32, tag="out")

            def post_fin(a, b):
                nn_r = b - a
                u2v = U2[:, a * D2:b * D2].rearrange("p (n j) -> p n j", j=D2)
                nc.vector.tensor_scalar_add(
                    z_t[:, a:b, None], u2v[:, :, D2 - 1:D2], 1e-16)
                nc.vector.reciprocal(rec_t[:, a:b], z_t[:, a:b])
                nc.vector.tensor_tensor(
                    out=o_t[:, a * OUT:b * OUT].rearrange(
                        "p (n j) -> p n j", j=OUT),
                    in0=u2v[:, :, 0:OUT],
                    in1=rec_t[:, a:b, None].to_broadcast([P, nn_r, OUT]),
                    op=OP.mult)
                nc.sync.dma_start(out=out_d[:, a * OUT:b * OUT],
                                  in_=o_t[:, a * OUT:b * OUT])

            edge_layer(tab2_d, D2, ad2_t, U2, post=post_fin)
    nc.finalize()
    return nc


def kernel(V, E, X, W1, a1_src, a1_dst, W2, a2_src, a2_dst):
    meta = _preprocess(E, X)
    nc = _build(meta)

    from concourse.bass_utils import run_bass_kernel_spmd

    in_maps = []
    for c in range(N_CORES):
        in_maps.append({
            "XcT": np.ascontiguousarray(meta.XcT[c]),
            "ones": np.ascontiguousarray(meta.ones[c]),
            "idx": np.ascontiguousarray(meta.idx[c]),
            "W1": np.asarray(W1, dtype=ml_dtypes.bfloat16),
            "a1s": np.asarray(a1_src, np.float32).reshape(1, -1),
            "a1d": np.asarray(a1_dst, np.float32).reshape(1, -1),
            "W2T": np.ascontiguousarray(np.asarray(W2, np.float32).T).reshape(1, -1),
            "a2s": np.asarray(a2_src, np.float32).reshape(1, -1),
            "a2d": np.asarray(a2_dst, np.float32).reshape(1, -1),
        })
    res = run_bass_kernel_spmd(nc, in_maps, list(range(N_CORES)))

    out = np.zeros((N_NODES, OUT), dtype=np.float32)
    for c in range(N_CORES):
        g = res.results[c]["out"].reshape(P, meta.nrow_tot, OUT)
        pp, ii = np.nonzero(meta.cell2node[c] >= 0)
        ls = meta.cell2node[c, pp, ii]
        out[meta.p2n[c * meta.NLOC + ls]] = g[pp, ii]
    return out

